# revision 9
# baseline (speedup 1.0000x reference)
"""Trainium2 Bass kernel for nn_DifferentialMaxtree (v3).

Strategy (8 NeuronCores, data-parallel over the 32 (b,n) trees, 4 per core):
  1. Features/logits/sigmoid/w computed in "slot" layout (host pre-permutes
     attrs/diff), one batched [128,512,17] mult+reduce for the logits.
  2. Pointer-doubling chase (host precomputes the per-iteration pointer
     chains).  Per iteration:
       - gather table [128, 8192] (16x replicated chunk layout) rebuilt
         ON-CHIP by 16 PE selection-matmuls (fp16 moving x 0/1 stationary
         -> PSUM, exact for fp16 values) + scalar-engine evacuation,
       - gpsimd ap_gather (8192 idx/core, ~1us),
       - one DVE 32x32 stream-transpose colocates each request's 8
         candidates onto its own partition,
       - masked select (gpsimd is_equal + DVE mult + DVE reduce) lands
         exactly in state layout -> elementwise add.
     No DRAM traffic at all inside the chase loop except the tiny idx/qsel
     streams.
  3. Pixel lookup: host sorts each tree's pixels by table chunk into
     fixed-size per-(round, chunk, core) runs, so the gather result is
     extracted by 8 contiguous DMAs per round straight to DRAM -- no
     transpose, no select.  Host inverts the sort during assembly.

Slot layout: slot (p, x) of a tree holds component
    e(p, x) = 8192*(2*(p//32) + x%2) + 512*(p%16) + (x & ~1) + (p//16)%2
which makes (a) the chase gather-idx tile natural-layout, (b) the select
output land in state layout, and (c) the table chunks contiguous.
"""

import numpy as np

import concourse.bass as bass
import concourse.bacc as bacc
import concourse.mybir as mybir
import concourse.tile as tile
from concourse.bass_utils import run_bass_kernel_spmd

f32 = mybir.dt.float32
f16 = mybir.dt.float16
i16 = mybir.dt.int16
u8 = mybir.dt.uint8
Alu = mybir.AluOpType
Act = mybir.ActivationFunctionType

CFG = dict(
    B=4, N=8, H=512, W=512, C=65536,
    NCORES=8,   # NeuronCores
    TPC=4,      # trees per NeuronCore
    P=128,
    EPS=1e-10,
    SCALING=10.0,
)

P = 128
C = 65536
J = 512          # free size of state image
NE = C // 8      # table row length (one chunk)
NPIX = 262144    # pixels per tree
NR = 5           # pixel rounds per tree


def _slot_component_map():
    """e(p, x): component id held at state slot (p, x)."""
    p = np.arange(P, dtype=np.int64)[:, None]
    x = np.arange(J, dtype=np.int64)[None, :]
    e = (8192 * (2 * (p // 32) + (x % 2)) + 512 * (p % 16)
         + (x & ~1) + (p // 16) % 2)
    return e  # [128, 512]


def _rem(g):
    """Within-chunk table offset of component/sentinel g (sentinel -> 0)."""
    g = g.astype(np.int64)
    return (512 * ((g // 512) % 16) + ((g % 512) & ~1)
            + (g // 8192) % 2).astype(np.int16)


def _chunk(g):
    """Table chunk of g in [0,8); sentinel C maps to 8 (selects nothing)."""
    g = g.astype(np.int64)
    return (2 * (g // 16384) + (g % 2)).astype(np.uint8)


def _sel_matrices():
    """Table-rebuild selection matrices (stationary operands for PE).

    M[0]: chase table, row q holds chunk q%8:   M[p,m,q] = (p == 16*(q%8)+m)
    M[1]: pixel table, row q holds chunk q//16: M[p,m,q] = (p == 16*(q//16)+m)
    """
    p = np.arange(P)[:, None, None]
    m = np.arange(16)[None, :, None]
    q = np.arange(P)[None, None, :]
    mc = (p == 16 * (q % 8) + m).astype(np.float16)
    mp = (p == 16 * (q // 16) + m).astype(np.float16)
    return np.stack([mc, mp])  # [2, 128, 16, 128]


# ---------------------------------------------------------------- host prep

HOST_STATE = {}


def _host_prep(cfg, diff, attrs, weight, bias, parent, pix2cc):
    B, N = cfg["B"], cfg["N"]
    NCORES, TPC = cfg["NCORES"], cfg["TPC"]

    e_slot = _slot_component_map()

    # pointer chains (host: pure index bookkeeping; all float math on device)
    pz = np.concatenate([parent, np.full((B, N, 1), C, np.int32)], axis=-1)
    chains = []
    cur = pz.copy()
    for k in range(17):
        if (cur == C).all():
            break
        chains.append(cur[..., :C].copy())
        cur = np.take_along_axis(cur, cur, axis=-1)
    k_iters = len(chains)
    if k_iters == 0:
        chains.append(pz[..., :C].copy())
        k_iters = 1

    # pixel rounds: core k handles chunk k's pixels, 8192 per round
    maxcnt = 0
    for b in range(B):
        for n in range(N):
            cnt = np.bincount(_chunk(pix2cc[b, n].reshape(-1)), minlength=8)
            maxcnt = max(maxcnt, int(cnt.max()))
    NR2 = -(-maxcnt // NE)           # rounds needed for the largest chunk

    Mmats = _sel_matrices()

    in_maps = []
    asg_all = []
    for core in range(NCORES):
        attrs_sw = np.empty((TPC, P, J, 15), np.float32)
        diff_sw = np.empty((TPC, P, J), np.float32)
        wgtB = np.empty((P, TPC, 17), np.float32)
        biasB = np.empty((P, TPC), np.float32)
        c_idx = np.empty((TPC, k_iters, P, J), np.int16)
        c_mask = np.empty((TPC, k_iters, P, J, 8), np.uint8)
        p_idx = np.zeros((TPC, NR2, P, J), np.int16)
        asg = np.full((TPC, NR2, 8, NE), -1, np.int64)
        for j in range(TPC):
            t = core * TPC + j
            b, n = t // N, t % N
            attrs_sw[j] = attrs[b, n][e_slot]
            diff_sw[j] = diff[b, n][e_slot]
            wgtB[:, j, :] = weight[n, :, 0][None, :]
            biasB[:, j] = bias[n, 0]
            for k in range(k_iters):
                g = chains[k][b, n]                      # by component, [C]
                c_idx[j, k] = _rem(g).reshape(P, J)      # natural layout
                c_mask[j, k] = (_chunk(g)[e_slot][:, :, None]
                                == np.arange(8, dtype=np.uint8))
            # ---- sorted pixels: core k <-> chunk k
            pix = pix2cc[b, n].reshape(-1)
            ch = _chunk(pix)
            rm = _rem(pix)
            order = np.argsort(ch, kind="stable")
            bnd = np.searchsorted(ch[order], np.arange(9))
            i_all = np.arange(NE, dtype=np.int64)
            rows = (i_all % 16)[None, :] + 16 * np.arange(8)[:, None]
            cols = (i_all // 16)[None, :].repeat(8, axis=0)
            for R in range(NR2):
                vals = np.zeros((8, NE), np.int16)       # [core, slot]
                for q in range(8):
                    qs, qe = bnd[q], bnd[q + 1]
                    lo = qs + R * NE
                    pick = order[lo:min(lo + NE, qe)]
                    npick = len(pick)
                    if npick == 0:
                        continue
                    vals[q, :npick] = rm[pick]
                    asg[j, R, q, :npick] = pick
                pt = np.zeros((P, J), np.int16)
                pt[rows, cols] = vals
                p_idx[j, R] = pt
        in_maps.append(dict(
            attrs_sw=attrs_sw, diff_sw=diff_sw, wgtB=wgtB, biasB=biasB,
            c_idx=c_idx, c_mask=c_mask, p_idx=p_idx,
            Mmats=Mmats,
        ))
        asg_all.append(asg)
    HOST_STATE["asg"] = asg_all
    return in_maps, k_iters, NR2


def _host_assemble(cfg, results):
    B, N = cfg["B"], cfg["N"]
    NCORES, TPC = cfg["NCORES"], cfg["TPC"]
    H, W = cfg["H"], cfg["W"]
    asg_all = HOST_STATE["asg"]
    out = np.empty((B, N, H * W), np.float32)
    for core in range(NCORES):
        ps = results[core]["pixsorted"]  # [TPC, NR2, 8, NE]
        asg = asg_all[core]
        for j in range(TPC):
            t = core * TPC + j
            b, n = t // N, t % N
            a = asg[j].reshape(-1)
            v = ps[j].reshape(-1)
            m = a >= 0
            out[b, n][a[m]] = v[m]
    return out.reshape(B, N, H, W)


# ------------------------------------------------------------- device build


def _build(cfg, k_iters, NR2):
    TPC, EPS = cfg["TPC"], cfg["EPS"]

    nc = bacc.Bacc("TRN2", target_bir_lowering=False, num_devices=cfg["NCORES"])
    attrs_sw = nc.dram_tensor("attrs_sw", [TPC, P, J, 15], f32, kind="ExternalInput")
    diff_sw = nc.dram_tensor("diff_sw", [TPC, P, J], f32, kind="ExternalInput")
    wgtB = nc.dram_tensor("wgtB", [P, TPC, 17], f32, kind="ExternalInput")
    biasB = nc.dram_tensor("biasB", [P, TPC], f32, kind="ExternalInput")
    c_idx = nc.dram_tensor("c_idx", [TPC, k_iters, P, J], i16, kind="ExternalInput")
    c_mask = nc.dram_tensor("c_mask", [TPC, k_iters, P, J, 8], u8,
                            kind="ExternalInput")
    p_idx = nc.dram_tensor("p_idx", [TPC, NR2, P, J], i16, kind="ExternalInput")
    Mmats = nc.dram_tensor("Mmats", [2, P, 16, P], f16, kind="ExternalInput")
    pixsorted = nc.dram_tensor("pixsorted", [TPC, NR2, 8, NE], f32,
                               kind="ExternalOutput")

    with tile.TileContext(nc) as tc:
        with (
            tc.tile_pool(name="sb", bufs=1) as pool0,
            tc.tile_pool(name="psum", bufs=1, space="PSUM") as ppool,
        ):
            wg = pool0.tile([P, TPC, 17], f32, tag="wg")
            nc.sync.dma_start(wg[:], wgtB[:, :, :])
            bi = pool0.tile([P, TPC], f32, tag="bi")
            nc.sync.dma_start(bi[:], biasB[:, :])
            mt = pool0.tile([P, 2, 16, P], f16, tag="mt")
            nc.sync.dma_start(mt[:], Mmats[:, :, :, :].rearrange("s p m q -> p s m q"))
            epsb = pool0.tile([P, 1], f32, tag="epsb")
            nc.vector.memset(epsb[:], 1e-10)
            hpib = pool0.tile([P, 1], f32, tag="hpib")
            nc.vector.memset(hpib[:], float(np.pi / 2))

            # ---------------- features -> s_j (= w of tree j), fp32
            s_tiles = []
            with tc.tile_pool(name="sbf", bufs=1) as poolf:
                for j in range(TPC):
                    at = poolf.tile([P, J, 15], f32, tag="at", bufs=2)
                    nc.sync.dma_start(at[:], attrs_sw[j])
                    df = poolf.tile([P, J], f32, tag="df", bufs=2)
                    nc.scalar.dma_start(df[:], diff_sw[j])
                    ft = poolf.tile([P, J, 17], f32, tag="ft")
                    t9 = poolf.tile([P, J, 9], f32, tag="t9")
                    t1 = poolf.tile([P, J], f32, tag="t1")
                    t2 = poolf.tile([P, J], f32, tag="t2")
                    lg = poolf.tile([P, J], f32, tag="lg")

                    # features 0..3: bbox copy
                    nc.scalar.activation(ft[:, :, 0:4], at[:, :, 0:4], Act.Copy)
                    # 4: log(area)
                    nc.scalar.activation(ft[:, :, 4], at[:, :, 4], Act.Ln)
                    # 5..13: log(|a|+eps)*sign(a) for raw attrs 6..14
                    nc.scalar.activation(t9[:], at[:, :, 6:15], Act.Abs)
                    nc.scalar.activation(t9[:], t9[:], Act.Ln, bias=epsb[:, :])
                    nc.scalar.activation(ft[:, :, 5:14], at[:, :, 6:15], Act.Sign)
                    nc.vector.tensor_tensor(
                        out=ft[:, :, 5:14], in0=ft[:, :, 5:14], in1=t9[:],
                        op=Alu.mult)
                    # 14: lshape = sqrt(a7) / (sqrt(a6) + eps)
                    nc.scalar.activation(t1[:], at[:, :, 6], Act.Sqrt)
                    nc.scalar.activation(t1[:], t1[:], Act.Copy, bias=EPS)
                    nc.vector.reciprocal(t1[:], t1[:])
                    nc.scalar.activation(t2[:], at[:, :, 7], Act.Sqrt)
                    nc.vector.tensor_tensor(
                        out=ft[:, :, 14], in0=t2[:], in1=t1[:], op=Alu.mult)
                    # 15: cos(angle), 16: sin(angle)
                    nc.scalar.activation(ft[:, :, 15], at[:, :, 5], Act.Sin,
                                         bias=hpib[:, :])
                    nc.scalar.activation(ft[:, :, 16], at[:, :, 5], Act.Sin)
                    # logits = <feats, w> + bias; sigmoid; w = diff * score
                    nc.vector.tensor_tensor(
                        out=ft[:], in0=ft[:],
                        in1=wg[:, j, :].unsqueeze(1).to_broadcast([P, J, 17]),
                        op=Alu.mult)
                    nc.vector.tensor_reduce(
                        out=lg[:], in_=ft[:], axis=mybir.AxisListType.X,
                        op=Alu.add)
                    nc.vector.tensor_tensor(
                        out=lg[:], in0=lg[:],
                        in1=bi[:, j:j + 1].to_broadcast([P, J]), op=Alu.add)
                    nc.scalar.activation(lg[:], lg[:], Act.Sigmoid)
                    s_j = pool0.tile([P, J], f32, tag=f"s{j}")
                    nc.vector.tensor_tensor(
                        out=s_j[:], in0=lg[:], in1=df[:], op=Alu.mult)
                    s_tiles.append(s_j)

            # ---------------- chase + pixels
            _es = __import__("contextlib").ExitStack()
            pool = _es.enter_context(tc.tile_pool(name="sbc", bufs=1))

            tbl_bufs = []
            for tb in range(2):
                tbl_b = pool.tile([P, NE], f32, tag=f"tbl{tb}")
                tbl_bufs.append(tbl_b)

            def rebuild_table(j, s_ap, mset):
                """state image [128,512] -> 16x replicated table via PE."""
                tbl = tbl_bufs[j % 2]
                s16 = pool.tile([P, J], f16, tag=f"s16_{j % 2}")
                nc.scalar.activation(s16[:], s_ap, Act.Copy)
                for half in range(4):
                    ps = ppool.tile([P, 4 * J], f32, tag="ps", bufs=2)
                    for mm in range(4):
                        m = 4 * half + mm
                        nc.tensor.matmul(
                            ps[:, J * mm:J * (mm + 1)], mt[:, mset, m, :],
                            s16[:], start=True, stop=True)
                    nc.scalar.activation(
                        tbl[:, 4 * J * half:4 * J * (half + 1)], ps[:],
                        Act.Copy)
                return tbl

            def chase_round(tbl, idx_ap, mask8_ap, s_ap):
                g = pool.tile([P, NE], f32, tag="g")
                nc.gpsimd.ap_gather(
                    out_ap=g[:], in_ap=tbl[:], idxs_ap=idx_ap,
                    channels=P, num_elems=NE, d=1, num_idxs=NE)
                gt = pool.tile([P, NE], f32, tag="gt")
                nc.vector.transpose(gt[:], g[:])
                mku = pool.tile([P, J, 8], u8, tag="mku", bufs=2)
                nc.scalar.dma_start(mku[:], mask8_ap)
                mk = pool.tile([P, J, 8], f32, tag="mk")
                nc.scalar.activation(mk[:], mku[:], Act.Copy)
                cand = gt[:].rearrange("p (c u q) -> p (c u) q", u=2, q=16)
                nc.vector.tensor_tensor(
                    out=mk[:], in0=mk[:], in1=cand[:, :, 0:8], op=Alu.mult)
                sel = pool.tile([P, J], f32, tag="sel", bufs=2)
                nc.vector.tensor_reduce(
                    out=sel[:], in_=mk[:],
                    axis=mybir.AxisListType.X, op=Alu.add)
                nc.vector.tensor_tensor(
                    out=s_ap, in0=s_ap, in1=sel[:], op=Alu.add)

            # chase: trees interleaved two-wide
            cidx_t = {}
            for pair in (range(0, 2), range(2, 4)):
                for j in pair:
                    ci = pool.tile([P, k_iters, J], i16, tag=f"ci{j % 2}")
                    nc.scalar.dma_start(
                        ci[:], c_idx[j].rearrange("k p f -> p k f"))
                    cidx_t[j] = ci
                for k in range(k_iters):
                    for j in pair:
                        tbl = rebuild_table(j, s_tiles[j][:], 0)
                        chase_round(tbl, cidx_t[j][:, k, :],
                                    c_mask[j, k], s_tiles[j][:])

            # pixels: val = s/SCALING; core k serves chunk k, NR2 rounds
            for pair in (range(0, 2), range(2, 4)):
                pix_t = {}
                for j in pair:
                    nc.vector.tensor_scalar_mul(
                        s_tiles[j][:], s_tiles[j][:], 1.0 / cfg["SCALING"])
                    pi = pool.tile([P, NR2, J], i16, tag=f"pi{j % 2}")
                    nc.scalar.dma_start(
                        pi[:], p_idx[j].rearrange("k p f -> p k f"))
                    pix_t[j] = pi
                for j in pair:
                    tbl = rebuild_table(j, s_tiles[j][:], 1)
                    for R in range(NR2):
                        g2 = pool.tile([P, NE], f32, tag="g")
                        nc.gpsimd.ap_gather(
                            out_ap=g2[:], in_ap=tbl[:],
                            idxs_ap=pix_t[j][:, R, :],
                            channels=P, num_elems=NE, d=1, num_idxs=NE)
                        for q in range(8):
                            eng = nc.sync if q % 2 == 0 else nc.scalar
                            eng.dma_start(pixsorted[j, R, q:q + 1],
                                          g2[16 * q:16 * q + 1, :])
            _es.close()

    nc.compile()
    return nc


_CACHE = {}
TRACE = False
LAST_RESULT = None


def _get_nc(cfg, k_iters, NR2):
    key = (k_iters, NR2)
    if key not in _CACHE:
        _CACHE[key] = _build(cfg, k_iters, NR2)
    return _CACHE[key]


def kernel(diff, attrs, weight, bias, parent, pix2cc):
    cfg = CFG
    diff = np.ascontiguousarray(np.asarray(diff, np.float32))
    attrs = np.ascontiguousarray(np.asarray(attrs, np.float32))
    weight = np.ascontiguousarray(np.asarray(weight, np.float32))
    bias = np.ascontiguousarray(np.asarray(bias, np.float32))
    parent = np.ascontiguousarray(np.asarray(parent, np.int32))
    pix2cc = np.ascontiguousarray(np.asarray(pix2cc, np.int32))

    in_maps, k_iters, NR2 = _host_prep(
        cfg, diff, attrs, weight, bias, parent, pix2cc)
    nc = _get_nc(cfg, k_iters, NR2)
    res = run_bass_kernel_spmd(
        nc, in_maps, core_ids=list(range(cfg["NCORES"])), trace=TRACE)
    global LAST_RESULT
    LAST_RESULT = res
    return _host_assemble(cfg, res.results)


# revision 13
# speedup vs baseline: 1.2191x; 1.2191x over previous
"""Trainium2 Bass kernel for nn_DifferentialMaxtree (v3).

Strategy (8 NeuronCores, data-parallel over the 32 (b,n) trees, 4 per core):
  1. Features/logits/sigmoid/w computed in "slot" layout (host pre-permutes
     attrs/diff), one batched [128,512,17] mult+reduce for the logits.
  2. Pointer-doubling chase (host precomputes the per-iteration pointer
     chains).  Per iteration:
       - gather table [128, 8192] (16x replicated chunk layout) rebuilt
         ON-CHIP by 16 PE selection-matmuls (fp16 moving x 0/1 stationary
         -> PSUM, exact for fp16 values) + scalar-engine evacuation,
       - gpsimd ap_gather (8192 idx/core, ~1us),
       - one DVE 32x32 stream-transpose colocates each request's 8
         candidates onto its own partition,
       - masked select (gpsimd is_equal + DVE mult + DVE reduce) lands
         exactly in state layout -> elementwise add.
     No DRAM traffic at all inside the chase loop except the tiny idx/qsel
     streams.
  3. Pixel lookup: host sorts each tree's pixels by table chunk into
     fixed-size per-(round, chunk, core) runs, so the gather result is
     extracted by 8 contiguous DMAs per round straight to DRAM -- no
     transpose, no select.  Host inverts the sort during assembly.

Slot layout: slot (p, x) of a tree holds component
    e(p, x) = 8192*(2*(p//32) + x%2) + 512*(p%16) + (x & ~1) + (p//16)%2
which makes (a) the chase gather-idx tile natural-layout, (b) the select
output land in state layout, and (c) the table chunks contiguous.
"""

import numpy as np

import concourse.bass as bass
import concourse.bacc as bacc
import concourse.mybir as mybir
import concourse.tile as tile
from concourse.bass_utils import run_bass_kernel_spmd

f32 = mybir.dt.float32
f16 = mybir.dt.float16
i16 = mybir.dt.int16
u8 = mybir.dt.uint8
Alu = mybir.AluOpType
Act = mybir.ActivationFunctionType

CFG = dict(
    B=4, N=8, H=512, W=512, C=65536,
    NCORES=8,   # NeuronCores
    TPC=4,      # trees per NeuronCore
    P=128,
    EPS=1e-10,
    SCALING=10.0,
)

P = 128
C = 65536
J = 512          # free size of state image
NE = C // 8      # table row length (one chunk)
NPIX = 262144    # pixels per tree
NR = 5           # pixel rounds per tree


def _slot_component_map():
    """e(p, x): component id held at state slot (p, x)."""
    p = np.arange(P, dtype=np.int64)[:, None]
    x = np.arange(J, dtype=np.int64)[None, :]
    e = (8192 * (2 * (p // 32) + (x % 2)) + 512 * (p % 16)
         + (x & ~1) + (p // 16) % 2)
    return e  # [128, 512]


def _rem(g):
    """Within-chunk table offset of component/sentinel g (sentinel -> 0)."""
    g = g.astype(np.int64)
    return (512 * ((g // 512) % 16) + ((g % 512) & ~1)
            + (g // 8192) % 2).astype(np.int16)


def _chunk(g):
    """Table chunk of g in [0,8); sentinel C maps to 8 (selects nothing)."""
    g = g.astype(np.int64)
    return (2 * (g // 16384) + (g % 2)).astype(np.uint8)


def _sel_matrices():
    """Table-rebuild selection matrices (stationary operands for PE).

    M[0]: chase table, row q holds chunk q%8:   M[p,m,q] = (p == 16*(q%8)+m)
    M[1]: pixel table, row q holds chunk q//16: M[p,m,q] = (p == 16*(q//16)+m)
    """
    p = np.arange(P)[:, None, None]
    m = np.arange(16)[None, :, None]
    q = np.arange(P)[None, None, :]
    mc = (p == 16 * (q % 8) + m).astype(np.float16)
    mp = (p == 16 * (q // 16) + m).astype(np.float16)
    return np.stack([mc, mp])  # [2, 128, 16, 128]


# ---------------------------------------------------------------- host prep

HOST_STATE = {}


def _host_prep(cfg, diff, attrs, weight, bias, parent, pix2cc):
    B, N = cfg["B"], cfg["N"]
    NCORES, TPC = cfg["NCORES"], cfg["TPC"]

    e_slot = _slot_component_map()
    # request index of slot (p, x): i = 32*(x//2) + p%32 (per core)
    _p = np.arange(P, dtype=np.int64)[:, None]
    _x = np.arange(J, dtype=np.int64)[None, :]
    req_of_slot = 32 * (_x // 2) + (_p % 32)

    # pointer chains (host: pure index bookkeeping; all float math on device)
    pz = np.concatenate([parent, np.full((B, N, 1), C, np.int32)], axis=-1)
    chains = []
    cur = pz.copy()
    for k in range(17):
        if (cur == C).all():
            break
        chains.append(cur[..., :C].copy())
        cur = np.take_along_axis(cur, cur, axis=-1)
    k_iters = len(chains)
    if k_iters == 0:
        chains.append(pz[..., :C].copy())
        k_iters = 1

    # per-iteration gather prefix sizes: slots are assigned per tree so that
    # components active longest come first (low request index).  NI[k] =
    # request-prefix needed to cover every component active at iteration k,
    # maxed over trees (one compiled kernel for all cores).
    ranks = np.zeros((B, N, C), np.int64)     # sort key: -last_active_iter
    for k in range(k_iters):
        ranks -= (chains[k] != C)
    NI = [0] * k_iters
    slot_comp = np.empty((B, N, P, J), np.int64)   # component at slot (p,x)
    slot_nat_all = np.empty((B, N, C), np.int64)   # component -> slot-space id
    _slot_nat_flat = e_slot.reshape(-1).copy()
    for b in range(B):
        for n in range(N):
            order = np.argsort(ranks[b, n], kind="stable")
            # rank r -> slot with request index i = r // 256? No: slots with
            # request i live at c2 = i//32; 256 slots share each c2 value.
            # Assign rank-major into (c2, p, u) so low ranks get low i.
            sc = np.empty((C,), np.int64)
            # slot enumeration in (c2, p, u) order == rank order
            c2r = np.arange(C) // 256
            rem256 = np.arange(C) % 256
            pr = rem256 // 2
            ur = rem256 % 2
            xr = 2 * c2r + ur
            sc = (pr, xr)
            grid = np.empty((P, J), np.int64)
            grid[pr, xr] = order
            slot_comp[b, n] = grid
            inv = np.empty((C,), np.int64)
            inv[order] = np.arange(C)
            slot_nat_all[b, n][grid.reshape(-1)] = _slot_nat_flat
            for k in range(k_iters):
                act = chains[k][b, n] != C
                if act.any():
                    worst = inv[act].max()
                    need = 32 * (-(-(worst + 1) // 256) * 8)  # ceil to c2 *256 slots -> 32*8*c2...
                    # requests covering c2 < ceil((worst+1)/256): i < 32*ceil(...)
                    nic = 32 * (-(-(worst + 1) // 256))
                    NI[k] = max(NI[k], int(nic))
    NI = [min(-(-v // 16) * 16, NE) if v > 0 else 16 for v in NI]
    # only the final iteration uses a small prefix gather (dedicated SBUF
    # tiles); earlier iterations are ~full anyway and share the big tiles
    NI = [NE] * (k_iters - 1) + [NI[-1] if NI[-1] <= NE // 4 else NE]

    # pixel rounds: core k handles chunk k's pixels (slot-space), 8192/round
    maxcnt = 0
    for b in range(B):
        for n in range(N):
            pxs = slot_nat_all[b, n][pix2cc[b, n].reshape(-1)]
            cnt = np.bincount(_chunk(pxs), minlength=8)
            maxcnt = max(maxcnt, int(cnt.max()))
    NR2 = -(-maxcnt // NE)           # rounds needed for the largest chunk

    Mmats = _sel_matrices()

    in_maps = []
    asg_all = []
    for core in range(NCORES):
        attrs_sw = np.empty((TPC, P, J, 15), np.float32)
        diff_sw = np.empty((TPC, P, J), np.float32)
        wgtB = np.empty((P, TPC, 17), np.float32)
        biasB = np.empty((P, TPC), np.float32)
        c_idx = np.empty((TPC, k_iters, P, J), np.int16)
        c_mask = np.empty((TPC, k_iters, P, J, 8), np.uint8)
        p_idx = np.zeros((TPC, NR2, P, J), np.int16)
        asg = np.full((TPC, NR2, 8, NE), -1, np.int64)
        for j in range(TPC):
            t = core * TPC + j
            b, n = t // N, t % N
            es = slot_comp[b, n]
            attrs_sw[j] = attrs[b, n][es]
            diff_sw[j] = diff[b, n][es]
            wgtB[:, j, :] = weight[n, :, 0][None, :]
            biasB[:, j] = bias[n, 0]
            # slot-space parent pointers: component ids relabeled by the
            # per-tree slot permutation (slot-space id of component c is its
            # slot's "natural" id e(p,x); table/idx geometry is slot-space).
            nat_of_slot = e_slot                          # e(p, x)
            slot_nat = slot_nat_all[b, n]
            for k in range(k_iters):
                g = chains[k][b, n]                      # by component, [C]
                gs = np.where(g == C, C, slot_nat[np.minimum(g, C - 1)])
                # idx tile natural layout in slot-space ids: position of the
                # requesting component's nat id
                ci = np.zeros((P, J), np.int16)
                ci.reshape(-1)[nat_of_slot.reshape(-1)] = _rem(
                    gs[slot_comp[b, n]].reshape(-1))
                c_idx[j, k] = ci
                c_mask[j, k] = (_chunk(gs[slot_comp[b, n]])[:, :, None]
                                == np.arange(8, dtype=np.uint8))
            # ---- sorted pixels: core k <-> chunk k (slot-space ids)
            pix = slot_nat[pix2cc[b, n].reshape(-1)]
            ch = _chunk(pix)
            rm = _rem(pix)
            order = np.argsort(ch, kind="stable")
            bnd = np.searchsorted(ch[order], np.arange(9))
            i_all = np.arange(NE, dtype=np.int64)
            rows = (i_all % 16)[None, :] + 16 * np.arange(8)[:, None]
            cols = (i_all // 16)[None, :].repeat(8, axis=0)
            for R in range(NR2):
                vals = np.zeros((8, NE), np.int16)       # [core, slot]
                for q in range(8):
                    qs, qe = bnd[q], bnd[q + 1]
                    lo = qs + R * NE
                    pick = order[lo:min(lo + NE, qe)]
                    npick = len(pick)
                    if npick == 0:
                        continue
                    vals[q, :npick] = rm[pick]
                    asg[j, R, q, :npick] = pick
                pt = np.zeros((P, J), np.int16)
                pt[rows, cols] = vals
                p_idx[j, R] = pt
        in_maps.append(dict(
            attrs_sw=attrs_sw, diff_sw=diff_sw, wgtB=wgtB, biasB=biasB,
            c_idx=c_idx, c_mask=c_mask, p_idx=p_idx,
            Mmats=Mmats,
        ))
        asg_all.append(asg)
    # last pixel round only needs the global max remainder
    NLAST = 16
    for core in range(NCORES):
        pass
    rem_needed = maxcnt - (NR2 - 1) * NE
    NLAST = max(16, min(NE, -(-rem_needed // 16) * 16))
    HOST_STATE["asg"] = asg_all
    return in_maps, k_iters, (NR2, NLAST, tuple(NI))


def _host_assemble(cfg, results):
    B, N = cfg["B"], cfg["N"]
    NCORES, TPC = cfg["NCORES"], cfg["TPC"]
    H, W = cfg["H"], cfg["W"]
    asg_all = HOST_STATE["asg"]
    out = np.empty((B, N, H * W), np.float32)
    for core in range(NCORES):
        ps = results[core]["pixsorted"]  # [TPC, NR2, 8, NE]
        asg = asg_all[core]
        for j in range(TPC):
            t = core * TPC + j
            b, n = t // N, t % N
            a = asg[j].reshape(-1)
            v = ps[j].reshape(-1)
            m = a >= 0
            out[b, n][a[m]] = v[m]
    return out.reshape(B, N, H, W)


# ------------------------------------------------------------- device build


def _build(cfg, k_iters, geom):
    TPC, EPS = cfg["TPC"], cfg["EPS"]
    NR2, NLAST, NI = geom

    nc = bacc.Bacc("TRN2", target_bir_lowering=False, num_devices=cfg["NCORES"])
    attrs_sw = nc.dram_tensor("attrs_sw", [TPC, P, J, 15], f32, kind="ExternalInput")
    diff_sw = nc.dram_tensor("diff_sw", [TPC, P, J], f32, kind="ExternalInput")
    wgtB = nc.dram_tensor("wgtB", [P, TPC, 17], f32, kind="ExternalInput")
    biasB = nc.dram_tensor("biasB", [P, TPC], f32, kind="ExternalInput")
    c_idx = nc.dram_tensor("c_idx", [TPC, k_iters, P, J], i16, kind="ExternalInput")
    c_mask = nc.dram_tensor("c_mask", [TPC, k_iters, P, J, 8], u8,
                            kind="ExternalInput")
    p_idx = nc.dram_tensor("p_idx", [TPC, NR2, P, J], i16, kind="ExternalInput")
    Mmats = nc.dram_tensor("Mmats", [2, P, 16, P], f16, kind="ExternalInput")
    pixsorted = nc.dram_tensor("pixsorted", [TPC, NR2, 8, NE], f32,
                               kind="ExternalOutput")

    with tile.TileContext(nc) as tc:
        with (
            tc.tile_pool(name="sb", bufs=1) as pool0,
            tc.tile_pool(name="psum", bufs=1, space="PSUM") as ppool,
        ):
            wg = pool0.tile([P, TPC, 17], f32, tag="wg")
            nc.sync.dma_start(wg[:], wgtB[:, :, :])
            bi = pool0.tile([P, TPC], f32, tag="bi")
            nc.sync.dma_start(bi[:], biasB[:, :])
            mt = pool0.tile([P, 2, 16, P], f16, tag="mt")
            nc.sync.dma_start(mt[:], Mmats[:, :, :, :].rearrange("s p m q -> p s m q"))
            epsb = pool0.tile([P, 1], f32, tag="epsb")
            nc.vector.memset(epsb[:], 1e-10)
            hpib = pool0.tile([P, 1], f32, tag="hpib")
            nc.vector.memset(hpib[:], float(np.pi / 2))

            # ---------------- features -> s_j (= w of tree j), fp32
            s_tiles = []
            with tc.tile_pool(name="sbf", bufs=1) as poolf:
                for j in range(TPC):
                    at = poolf.tile([P, J, 15], f32, tag="at", bufs=2)
                    nc.sync.dma_start(at[:], attrs_sw[j])
                    df = poolf.tile([P, J], f32, tag="df", bufs=2)
                    nc.scalar.dma_start(df[:], diff_sw[j])
                    ft = poolf.tile([P, J, 17], f32, tag="ft")
                    t9 = poolf.tile([P, J, 9], f32, tag="t9")
                    t1 = poolf.tile([P, J], f32, tag="t1")
                    t2 = poolf.tile([P, J], f32, tag="t2")
                    lg = poolf.tile([P, J], f32, tag="lg")

                    # features 0..3: bbox copy
                    nc.scalar.activation(ft[:, :, 0:4], at[:, :, 0:4], Act.Copy)
                    # 4: log(area)
                    nc.scalar.activation(ft[:, :, 4], at[:, :, 4], Act.Ln)
                    # 5..13: log(|a|+eps)*sign(a) for raw attrs 6..14
                    nc.scalar.activation(t9[:], at[:, :, 6:15], Act.Abs)
                    nc.scalar.activation(t9[:], t9[:], Act.Ln, bias=epsb[:, :])
                    nc.scalar.activation(ft[:, :, 5:14], at[:, :, 6:15], Act.Sign)
                    nc.vector.tensor_tensor(
                        out=ft[:, :, 5:14], in0=ft[:, :, 5:14], in1=t9[:],
                        op=Alu.mult)
                    # 14: lshape = sqrt(a7) / (sqrt(a6) + eps)
                    nc.scalar.activation(t1[:], at[:, :, 6], Act.Sqrt)
                    nc.scalar.activation(t1[:], t1[:], Act.Copy, bias=EPS)
                    nc.vector.reciprocal(t1[:], t1[:])
                    nc.scalar.activation(t2[:], at[:, :, 7], Act.Sqrt)
                    nc.vector.tensor_tensor(
                        out=ft[:, :, 14], in0=t2[:], in1=t1[:], op=Alu.mult)
                    # 15: cos(angle), 16: sin(angle)
                    nc.scalar.activation(ft[:, :, 15], at[:, :, 5], Act.Sin,
                                         bias=hpib[:, :])
                    nc.scalar.activation(ft[:, :, 16], at[:, :, 5], Act.Sin)
                    # logits = <feats, w> + bias; sigmoid; w = diff * score
                    nc.vector.tensor_tensor(
                        out=ft[:], in0=ft[:],
                        in1=wg[:, j, :].unsqueeze(1).to_broadcast([P, J, 17]),
                        op=Alu.mult)
                    nc.vector.tensor_reduce(
                        out=lg[:], in_=ft[:], axis=mybir.AxisListType.X,
                        op=Alu.add)
                    nc.vector.tensor_tensor(
                        out=lg[:], in0=lg[:],
                        in1=bi[:, j:j + 1].to_broadcast([P, J]), op=Alu.add)
                    nc.scalar.activation(lg[:], lg[:], Act.Sigmoid)
                    s_j = pool0.tile([P, J], f32, tag=f"s{j}")
                    nc.vector.tensor_tensor(
                        out=s_j[:], in0=lg[:], in1=df[:], op=Alu.mult)
                    s_tiles.append(s_j)

            # ---------------- chase + pixels
            _es = __import__("contextlib").ExitStack()
            pool = _es.enter_context(tc.tile_pool(name="sbc", bufs=1))

            tbl_bufs = []
            for tb in range(2):
                tbl_b = pool.tile([P, NE], f32, tag=f"tbl{tb}")
                tbl_bufs.append(tbl_b)

            def rebuild_table(j, s_ap, mset):
                """state image [128,512] -> 16x replicated table via PE."""
                tbl = tbl_bufs[j % 2]
                s16 = pool.tile([P, J], f16, tag=f"s16_{j % 2}")
                nc.scalar.activation(s16[:], s_ap, Act.Copy)
                for half in range(4):
                    ps = ppool.tile([P, 4 * J], f32, tag="ps", bufs=2)
                    for mm in range(4):
                        m = 4 * half + mm
                        nc.tensor.matmul(
                            ps[:, J * mm:J * (mm + 1)], mt[:, mset, m, :],
                            s16[:], start=True, stop=True)
                    nc.scalar.activation(
                        tbl[:, 4 * J * half:4 * J * (half + 1)], ps[:],
                        Act.Copy)
                return tbl

            def chase_round(tbl, idx_ap, mask8_ap, s_ap, ni, tagsfx=""):
                xs = ni // 16            # slots-per-partition covered
                g = pool.tile([P, ni], f32, tag="g" + tagsfx)
                nc.gpsimd.ap_gather(
                    out_ap=g[:], in_ap=tbl[:], idxs_ap=idx_ap,
                    channels=P, num_elems=NE, d=1, num_idxs=ni)
                gt = pool.tile([P, ni], f32, tag="gt" + tagsfx)
                nc.vector.transpose(gt[:], g[:])
                mku = pool.tile([P, xs, 8], u8, tag="mku" + tagsfx, bufs=1)
                nc.scalar.dma_start(mku[:], mask8_ap)
                mk = pool.tile([P, xs, 8], f32, tag="mk" + tagsfx)
                nc.scalar.activation(mk[:], mku[:], Act.Copy)
                cand = gt[:].rearrange("p (c u q) -> p (c u) q", u=2, q=16)
                nc.vector.tensor_tensor(
                    out=mk[:], in0=mk[:], in1=cand[:, :, 0:8], op=Alu.mult)
                sel = pool.tile([P, xs], f32, tag="sel" + tagsfx, bufs=2)
                nc.vector.tensor_reduce(
                    out=sel[:], in_=mk[:],
                    axis=mybir.AxisListType.X, op=Alu.add)
                nc.vector.tensor_tensor(
                    out=s_ap[:, 0:xs], in0=s_ap[:, 0:xs], in1=sel[:],
                    op=Alu.add)

            # chase: trees interleaved two-wide
            cidx_t = {}
            for pair in (range(0, 2), range(2, 4)):
                for j in pair:
                    ci = pool.tile([P, k_iters, J], i16, tag=f"ci{j % 2}")
                    nc.scalar.dma_start(
                        ci[:], c_idx[j].rearrange("k p f -> p k f"))
                    cidx_t[j] = ci
                for k in range(k_iters):
                    ni = NI[k]
                    sfx = "" if ni == NE else f"n{ni}"
                    for j in pair:
                        tbl = rebuild_table(j, s_tiles[j][:], 0)
                        chase_round(tbl, cidx_t[j][:, k, 0:ni // 16],
                                    c_mask[j, k, :, 0:ni // 16, :],
                                    s_tiles[j][:], ni, sfx)

            # pixels: val = s/SCALING; core k serves chunk k, NR2 rounds
            for pair in (range(0, 2), range(2, 4)):
                pix_t = {}
                for j in pair:
                    nc.vector.tensor_scalar_mul(
                        s_tiles[j][:], s_tiles[j][:], 1.0 / cfg["SCALING"])
                    pi = pool.tile([P, NR2, J], i16, tag=f"pi{j % 2}")
                    nc.scalar.dma_start(
                        pi[:], p_idx[j].rearrange("k p f -> p k f"))
                    pix_t[j] = pi
                for j in pair:
                    tbl = rebuild_table(j, s_tiles[j][:], 1)
                    for R in range(NR2):
                        last = R == NR2 - 1
                        ni = NLAST if last else NE
                        g2 = pool.tile([P, ni], f32,
                                       tag="g" if not last else "gpl")
                        nc.gpsimd.ap_gather(
                            out_ap=g2[:], in_ap=tbl[:],
                            idxs_ap=pix_t[j][:, R, 0:ni // 16],
                            channels=P, num_elems=NE, d=1, num_idxs=ni)
                        for q in range(8):
                            eng = nc.sync if q % 2 == 0 else nc.scalar
                            eng.dma_start(pixsorted[j, R, q:q + 1, 0:ni],
                                          g2[16 * q:16 * q + 1, :])
            _es.close()

    nc.compile()
    return nc


_CACHE = {}
TRACE = False
LAST_RESULT = None


def _get_nc(cfg, k_iters, geom):
    key = (k_iters, geom)
    if key not in _CACHE:
        _CACHE[key] = _build(cfg, k_iters, geom)
    return _CACHE[key]


def kernel(diff, attrs, weight, bias, parent, pix2cc):
    cfg = CFG
    diff = np.ascontiguousarray(np.asarray(diff, np.float32))
    attrs = np.ascontiguousarray(np.asarray(attrs, np.float32))
    weight = np.ascontiguousarray(np.asarray(weight, np.float32))
    bias = np.ascontiguousarray(np.asarray(bias, np.float32))
    parent = np.ascontiguousarray(np.asarray(parent, np.int32))
    pix2cc = np.ascontiguousarray(np.asarray(pix2cc, np.int32))

    in_maps, k_iters, geom = _host_prep(
        cfg, diff, attrs, weight, bias, parent, pix2cc)
    nc = _get_nc(cfg, k_iters, geom)
    res = run_bass_kernel_spmd(
        nc, in_maps, core_ids=list(range(cfg["NCORES"])), trace=TRACE)
    global LAST_RESULT
    LAST_RESULT = res
    return _host_assemble(cfg, res.results)


# revision 15
# speedup vs baseline: 1.4678x; 1.2040x over previous
"""Trainium2 Bass kernel for nn_DifferentialMaxtree (v3).

Strategy (8 NeuronCores, data-parallel over the 32 (b,n) trees, 4 per core):
  1. Features/logits/sigmoid/w computed in "slot" layout (host pre-permutes
     attrs/diff), one batched [128,512,17] mult+reduce for the logits.
  2. Pointer-doubling chase (host precomputes the per-iteration pointer
     chains).  Per iteration:
       - gather table [128, 8192] (16x replicated chunk layout) rebuilt
         ON-CHIP by 16 PE selection-matmuls (fp16 moving x 0/1 stationary
         -> PSUM, exact for fp16 values) + scalar-engine evacuation,
       - gpsimd ap_gather (8192 idx/core, ~1us),
       - one DVE 32x32 stream-transpose colocates each request's 8
         candidates onto its own partition,
       - masked select (gpsimd is_equal + DVE mult + DVE reduce) lands
         exactly in state layout -> elementwise add.
     No DRAM traffic at all inside the chase loop except the tiny idx/qsel
     streams.
  3. Pixel lookup: host sorts each tree's pixels by table chunk into
     fixed-size per-(round, chunk, core) runs, so the gather result is
     extracted by 8 contiguous DMAs per round straight to DRAM -- no
     transpose, no select.  Host inverts the sort during assembly.

Slot layout: slot (p, x) of a tree holds component
    e(p, x) = 8192*(2*(p//32) + x%2) + 512*(p%16) + (x & ~1) + (p//16)%2
which makes (a) the chase gather-idx tile natural-layout, (b) the select
output land in state layout, and (c) the table chunks contiguous.
"""

import numpy as np

import concourse.bass as bass
import concourse.bacc as bacc
import concourse.mybir as mybir
import concourse.tile as tile
from concourse.bass_utils import run_bass_kernel_spmd

f32 = mybir.dt.float32
f16 = mybir.dt.float16
i16 = mybir.dt.int16
u8 = mybir.dt.uint8
Alu = mybir.AluOpType
Act = mybir.ActivationFunctionType

CFG = dict(
    B=4, N=8, H=512, W=512, C=65536,
    NCORES=8,   # NeuronCores
    TPC=4,      # trees per NeuronCore
    P=128,
    EPS=1e-10,
    SCALING=10.0,
)

P = 128
C = 65536
J = 512          # free size of state image
NE = C // 8      # table row length (one chunk)
NPIX = 262144    # pixels per tree
NR = 5           # pixel rounds per tree


def _slot_component_map():
    """e(p, x): component id held at state slot (p, x)."""
    p = np.arange(P, dtype=np.int64)[:, None]
    x = np.arange(J, dtype=np.int64)[None, :]
    e = (8192 * (2 * (p // 32) + (x % 2)) + 512 * (p % 16)
         + (x & ~1) + (p // 16) % 2)
    return e  # [128, 512]


def _rem(g):
    """Within-chunk table offset of component/sentinel g (sentinel -> 0)."""
    g = g.astype(np.int64)
    return (512 * ((g // 512) % 16) + ((g % 512) & ~1)
            + (g // 8192) % 2).astype(np.int16)


def _chunk(g):
    """Table chunk of g in [0,8); sentinel C maps to 8 (selects nothing)."""
    g = g.astype(np.int64)
    return (2 * (g // 16384) + (g % 2)).astype(np.uint8)


def _sel_matrices():
    """Table-rebuild selection matrices (stationary operands for PE).

    M[0]: chase table, row q holds chunk q%8:   M[p,m,q] = (p == 16*(q%8)+m)
    M[1]: pixel table, row q holds chunk q//16: M[p,m,q] = (p == 16*(q//16)+m)
    """
    p = np.arange(P)[:, None, None]
    m = np.arange(16)[None, :, None]
    q = np.arange(P)[None, None, :]
    mc = (p == 16 * (q % 8) + m).astype(np.float16)
    mp = (p == 16 * (q // 16) + m).astype(np.float16)
    return np.stack([mc, mp])  # [2, 128, 16, 128]


# ---------------------------------------------------------------- host prep

HOST_STATE = {}


def _host_prep(cfg, diff, attrs, weight, bias, parent, pix2cc):
    B, N = cfg["B"], cfg["N"]
    NCORES, TPC = cfg["NCORES"], cfg["TPC"]

    e_slot = _slot_component_map()
    # request index of slot (p, x): i = 32*(x//2) + p%32 (per core)
    _p = np.arange(P, dtype=np.int64)[:, None]
    _x = np.arange(J, dtype=np.int64)[None, :]
    req_of_slot = 32 * (_x // 2) + (_p % 32)

    # pointer chains (host: pure index bookkeeping; all float math on device)
    pz = np.concatenate([parent, np.full((B, N, 1), C, np.int32)], axis=-1)
    chains = []
    cur = pz.copy()
    for k in range(17):
        if (cur == C).all():
            break
        chains.append(cur[..., :C].copy())
        cur = np.take_along_axis(cur, cur, axis=-1)
    k_iters = len(chains)
    if k_iters == 0:
        chains.append(pz[..., :C].copy())
        k_iters = 1

    # per-iteration gather prefix sizes: slots are assigned per tree so that
    # components active longest come first (low request index).  NI[k] =
    # request-prefix needed to cover every component active at iteration k,
    # maxed over trees (one compiled kernel for all cores).
    ranks = np.zeros((B, N, C), np.int64)     # sort key: -last_active_iter
    for k in range(k_iters):
        ranks -= (chains[k] != C)
    NI = [0] * k_iters
    slot_comp = np.empty((B, N, P, J), np.int64)   # component at slot (p,x)
    slot_nat_all = np.empty((B, N, C), np.int64)   # component -> slot-space id
    _slot_nat_flat = e_slot.reshape(-1).copy()
    for b in range(B):
        for n in range(N):
            order = np.argsort(ranks[b, n], kind="stable")
            # rank r -> slot with request index i = r // 256? No: slots with
            # request i live at c2 = i//32; 256 slots share each c2 value.
            # Assign rank-major into (c2, p, u) so low ranks get low i.
            sc = np.empty((C,), np.int64)
            # slot enumeration in (c2, p, u) order == rank order
            c2r = np.arange(C) // 256
            rem256 = np.arange(C) % 256
            pr = rem256 // 2
            ur = rem256 % 2
            xr = 2 * c2r + ur
            sc = (pr, xr)
            grid = np.empty((P, J), np.int64)
            grid[pr, xr] = order
            slot_comp[b, n] = grid
            inv = np.empty((C,), np.int64)
            inv[order] = np.arange(C)
            slot_nat_all[b, n][grid.reshape(-1)] = _slot_nat_flat
            for k in range(k_iters):
                act = chains[k][b, n] != C
                if act.any():
                    worst = inv[act].max()
                    need = 32 * (-(-(worst + 1) // 256) * 8)  # ceil to c2 *256 slots -> 32*8*c2...
                    # requests covering c2 < ceil((worst+1)/256): i < 32*ceil(...)
                    nic = 32 * (-(-(worst + 1) // 256))
                    NI[k] = max(NI[k], int(nic))
    NI = [min(-(-v // 16) * 16, NE) if v > 0 else 16 for v in NI]
    # only the final iteration uses a small prefix gather (dedicated SBUF
    # tiles); earlier iterations are ~full anyway and share the big tiles
    NI = [NE] * (k_iters - 1) + [NI[-1] if NI[-1] <= NE // 4 else NE]

    # pixel rounds: core k handles chunk k's pixels (slot-space).  Pixels of
    # a chunk are paired (even-rem, odd-rem) within each 2-entry fp16 table
    # bucket so one d=2 gather index serves two pixels.  Idx-slot count per
    # (tree, chunk) = sum over buckets of max(#even, #odd).
    maxslots = 0
    for b in range(B):
        for n in range(N):
            pxs = slot_nat_all[b, n][pix2cc[b, n].reshape(-1)]
            rm = _rem(pxs).astype(np.int64) + 8192 * _chunk(pxs).astype(np.int64)
            cnt = np.bincount(rm, minlength=8 * 8192)
            ev = cnt[0::2].reshape(8, 4096)
            od = cnt[1::2].reshape(8, 4096)
            slots = np.maximum(ev, od).sum(axis=1)
            maxslots = max(maxslots, int(slots.max()))
    NR2 = 4                          # paired rounds per tree
    NIPX = -(-maxslots // (NR2 * 16)) * 16   # idx per core per round

    Mmats = _sel_matrices()

    in_maps = []
    asg_all = []
    for core in range(NCORES):
        attrs_sw = np.empty((TPC, P, J, 15), np.float32)
        diff_sw = np.empty((TPC, P, J), np.float32)
        wgtB = np.empty((P, TPC, 17), np.float32)
        biasB = np.empty((P, TPC), np.float32)
        c_idx = np.empty((TPC, k_iters, P, J), np.int16)
        c_mask = np.empty((TPC, k_iters, P, J, 8), np.uint8)
        p_idx = np.zeros((TPC, NR2, P, NIPX // 16), np.int16)
        asg = np.full((TPC, NR2, 8, 2 * NIPX), -1, np.int64)
        for j in range(TPC):
            t = core * TPC + j
            b, n = t // N, t % N
            es = slot_comp[b, n]
            attrs_sw[j] = attrs[b, n][es]
            diff_sw[j] = diff[b, n][es]
            wgtB[:, j, :] = weight[n, :, 0][None, :]
            biasB[:, j] = bias[n, 0]
            # slot-space parent pointers: component ids relabeled by the
            # per-tree slot permutation (slot-space id of component c is its
            # slot's "natural" id e(p,x); table/idx geometry is slot-space).
            nat_of_slot = e_slot                          # e(p, x)
            slot_nat = slot_nat_all[b, n]
            for k in range(k_iters):
                g = chains[k][b, n]                      # by component, [C]
                gs = np.where(g == C, C, slot_nat[np.minimum(g, C - 1)])
                # idx tile natural layout in slot-space ids: position of the
                # requesting component's nat id
                ci = np.zeros((P, J), np.int16)
                ci.reshape(-1)[nat_of_slot.reshape(-1)] = _rem(
                    gs[slot_comp[b, n]].reshape(-1))
                c_idx[j, k] = ci
                c_mask[j, k] = (_chunk(gs[slot_comp[b, n]])[:, :, None]
                                == np.arange(8, dtype=np.uint8))
            # ---- paired sorted pixels: core k <-> chunk k (slot-space)
            pix = slot_nat[pix2cc[b, n].reshape(-1)]
            ch = _chunk(pix)
            rm = _rem(pix).astype(np.int64)
            key = rm + 8192 * ch.astype(np.int64)
            order = np.argsort(key, kind="stable")       # by (chunk, rem)
            ks = key[order]
            i_all = np.arange(NIPX, dtype=np.int64)
            rows = (i_all % 16)[None, :] + 16 * np.arange(8)[:, None]
            cols = (i_all // 16)[None, :].repeat(8, axis=0)
            # per chunk: build idx-slot stream (bucket m, lane assignment)
            idx_streams = []
            a_streams = []
            for q in range(8):
                lo, hi = np.searchsorted(ks, [q * 8192, (q + 1) * 8192])
                pr = ks[lo:hi] - q * 8192                # rems, sorted
                pid = order[lo:hi]
                mb = pr // 2
                par = pr % 2
                # rank within (bucket, parity)
                t = np.zeros(len(pr), np.int64)
                if len(pr):
                    new = np.ones(len(pr), bool)
                    samekey = (pr[1:] == pr[:-1])
                    new[1:] = ~samekey
                    grp = np.cumsum(new) - 1
                    first = np.zeros(grp[-1] + 1 if len(grp) else 0, np.int64)
                    np.add.at(first, grp, 0)
                    idxs_in_grp = np.arange(len(pr)) - np.searchsorted(
                        grp, grp, side="left")
                    t = idxs_in_grp
                # slot id within the chunk stream for (bucket, rank t)
                nslot = np.zeros(4096, np.int64)
                np.maximum.at(nslot, mb, t + 1)
                base = np.zeros(4096, np.int64)
                base[1:] = np.cumsum(nslot)[:-1]
                slot = base[mb] + t
                stream_len = int(nslot.sum())
                sidx = np.zeros(stream_len, np.int16)
                sidx[slot] = mb.astype(np.int16)
                sa = np.full(2 * stream_len, -1, np.int64)
                sa[2 * slot + par] = pid
                idx_streams.append(sidx)
                a_streams.append(sa)
            for R in range(NR2):
                vals = np.zeros((8, NIPX), np.int16)
                for q in range(8):
                    sidx = idx_streams[q][R * NIPX:(R + 1) * NIPX]
                    vals[q, :len(sidx)] = sidx
                    sa = a_streams[q][2 * R * NIPX:2 * (R + 1) * NIPX]
                    asg[j, R, q, :len(sa)] = sa
                pt = np.zeros((P, NIPX // 16), np.int16)
                pt[rows, cols] = vals
                p_idx[j, R] = pt
        in_maps.append(dict(
            attrs_sw=attrs_sw, diff_sw=diff_sw, wgtB=wgtB, biasB=biasB,
            c_idx=c_idx, c_mask=c_mask, p_idx=p_idx,
            Mmats=Mmats,
        ))
        asg_all.append(asg)
    HOST_STATE["asg"] = asg_all
    return in_maps, k_iters, (NR2, NIPX, tuple(NI))


def _host_assemble(cfg, results):
    B, N = cfg["B"], cfg["N"]
    NCORES, TPC = cfg["NCORES"], cfg["TPC"]
    H, W = cfg["H"], cfg["W"]
    asg_all = HOST_STATE["asg"]
    out = np.empty((B, N, H * W), np.float32)
    for core in range(NCORES):
        ps = results[core]["pixsorted"].astype(np.float32)
        asg = asg_all[core]
        for j in range(TPC):
            t = core * TPC + j
            b, n = t // N, t % N
            a = asg[j].reshape(-1)
            v = ps[j].reshape(-1)
            m = a >= 0
            out[b, n][a[m]] = v[m]
    return out.reshape(B, N, H, W)


# ------------------------------------------------------------- device build


def _build(cfg, k_iters, geom):
    TPC, EPS = cfg["TPC"], cfg["EPS"]
    NR2, NIPX, NI = geom

    nc = bacc.Bacc("TRN2", target_bir_lowering=False, num_devices=cfg["NCORES"])
    attrs_sw = nc.dram_tensor("attrs_sw", [TPC, P, J, 15], f32, kind="ExternalInput")
    diff_sw = nc.dram_tensor("diff_sw", [TPC, P, J], f32, kind="ExternalInput")
    wgtB = nc.dram_tensor("wgtB", [P, TPC, 17], f32, kind="ExternalInput")
    biasB = nc.dram_tensor("biasB", [P, TPC], f32, kind="ExternalInput")
    c_idx = nc.dram_tensor("c_idx", [TPC, k_iters, P, J], i16, kind="ExternalInput")
    c_mask = nc.dram_tensor("c_mask", [TPC, k_iters, P, J, 8], u8,
                            kind="ExternalInput")
    p_idx = nc.dram_tensor("p_idx", [TPC, NR2, P, NIPX // 16], i16,
                           kind="ExternalInput")
    Mmats = nc.dram_tensor("Mmats", [2, P, 16, P], f16, kind="ExternalInput")
    pixsorted = nc.dram_tensor("pixsorted", [TPC, NR2, 8, 2 * NIPX], f16,
                               kind="ExternalOutput")

    with tile.TileContext(nc) as tc:
        with (
            tc.tile_pool(name="sb", bufs=1) as pool0,
            tc.tile_pool(name="psum", bufs=1, space="PSUM") as ppool,
        ):
            wg = pool0.tile([P, TPC, 17], f32, tag="wg")
            nc.sync.dma_start(wg[:], wgtB[:, :, :])
            bi = pool0.tile([P, TPC], f32, tag="bi")
            nc.sync.dma_start(bi[:], biasB[:, :])
            mt = pool0.tile([P, 2, 16, P], f16, tag="mt")
            nc.sync.dma_start(mt[:], Mmats[:, :, :, :].rearrange("s p m q -> p s m q"))
            epsb = pool0.tile([P, 1], f32, tag="epsb")
            nc.vector.memset(epsb[:], 1e-10)
            hpib = pool0.tile([P, 1], f32, tag="hpib")
            nc.vector.memset(hpib[:], float(np.pi / 2))

            # ---------------- features -> s_j (= w of tree j), fp32
            s_tiles = []
            with tc.tile_pool(name="sbf", bufs=1) as poolf:
                for j in range(TPC):
                    at = poolf.tile([P, J, 15], f32, tag="at", bufs=2)
                    nc.sync.dma_start(at[:], attrs_sw[j])
                    df = poolf.tile([P, J], f32, tag="df", bufs=2)
                    nc.scalar.dma_start(df[:], diff_sw[j])
                    ft = poolf.tile([P, J, 17], f32, tag="ft")
                    t9 = poolf.tile([P, J, 9], f32, tag="t9")
                    t1 = poolf.tile([P, J], f32, tag="t1")
                    t2 = poolf.tile([P, J], f32, tag="t2")
                    lg = poolf.tile([P, J], f32, tag="lg")

                    # features 0..3: bbox copy
                    nc.scalar.activation(ft[:, :, 0:4], at[:, :, 0:4], Act.Copy)
                    # 4: log(area)
                    nc.scalar.activation(ft[:, :, 4], at[:, :, 4], Act.Ln)
                    # 5..13: log(|a|+eps)*sign(a) for raw attrs 6..14
                    nc.scalar.activation(t9[:], at[:, :, 6:15], Act.Abs)
                    nc.scalar.activation(t9[:], t9[:], Act.Ln, bias=epsb[:, :])
                    nc.scalar.activation(ft[:, :, 5:14], at[:, :, 6:15], Act.Sign)
                    nc.vector.tensor_tensor(
                        out=ft[:, :, 5:14], in0=ft[:, :, 5:14], in1=t9[:],
                        op=Alu.mult)
                    # 14: lshape = sqrt(a7) / (sqrt(a6) + eps)
                    nc.scalar.activation(t1[:], at[:, :, 6], Act.Sqrt)
                    nc.scalar.activation(t1[:], t1[:], Act.Copy, bias=EPS)
                    nc.vector.reciprocal(t1[:], t1[:])
                    nc.scalar.activation(t2[:], at[:, :, 7], Act.Sqrt)
                    nc.vector.tensor_tensor(
                        out=ft[:, :, 14], in0=t2[:], in1=t1[:], op=Alu.mult)
                    # 15: cos(angle), 16: sin(angle)
                    nc.scalar.activation(ft[:, :, 15], at[:, :, 5], Act.Sin,
                                         bias=hpib[:, :])
                    nc.scalar.activation(ft[:, :, 16], at[:, :, 5], Act.Sin)
                    # logits = <feats, w> + bias; sigmoid; w = diff * score
                    nc.vector.tensor_tensor(
                        out=ft[:], in0=ft[:],
                        in1=wg[:, j, :].unsqueeze(1).to_broadcast([P, J, 17]),
                        op=Alu.mult)
                    nc.vector.tensor_reduce(
                        out=lg[:], in_=ft[:], axis=mybir.AxisListType.X,
                        op=Alu.add)
                    nc.vector.tensor_tensor(
                        out=lg[:], in0=lg[:],
                        in1=bi[:, j:j + 1].to_broadcast([P, J]), op=Alu.add)
                    nc.scalar.activation(lg[:], lg[:], Act.Sigmoid)
                    s_j = pool0.tile([P, J], f32, tag=f"s{j}")
                    nc.vector.tensor_tensor(
                        out=s_j[:], in0=lg[:], in1=df[:], op=Alu.mult)
                    s_tiles.append(s_j)

            # ---------------- chase + pixels
            _es = __import__("contextlib").ExitStack()
            pool = _es.enter_context(tc.tile_pool(name="sbc", bufs=1))

            tbl_bufs = []
            for tb in range(2):
                tbl_b = pool.tile([P, NE], f32, tag=f"tbl{tb}")
                tbl_bufs.append(tbl_b)

            def rebuild_table(j, s_ap, mset):
                """state image [128,512] -> 16x replicated table via PE."""
                tbl = tbl_bufs[j % 2]
                s16 = pool.tile([P, J], f16, tag=f"s16_{j % 2}")
                nc.scalar.activation(s16[:], s_ap, Act.Copy)
                for half in range(4):
                    ps = ppool.tile([P, 4 * J], f32, tag="ps", bufs=2)
                    for mm in range(4):
                        m = 4 * half + mm
                        nc.tensor.matmul(
                            ps[:, J * mm:J * (mm + 1)], mt[:, mset, m, :],
                            s16[:], start=True, stop=True)
                    if mset == 0:
                        nc.scalar.activation(
                            tbl[:, 4 * J * half:4 * J * (half + 1)], ps[:],
                            Act.Copy)
                    else:
                        v16 = tbl[:].bitcast(f16)
                        nc.scalar.activation(
                            v16[:, 4 * J * half:4 * J * (half + 1)], ps[:],
                            Act.Copy)
                return tbl

            def chase_round(tbl, idx_ap, mask8_ap, s_ap, ni, tagsfx=""):
                xs = ni // 16            # slots-per-partition covered
                g = pool.tile([P, ni], f32, tag="g" + tagsfx)
                nc.gpsimd.ap_gather(
                    out_ap=g[:], in_ap=tbl[:], idxs_ap=idx_ap,
                    channels=P, num_elems=NE, d=1, num_idxs=ni)
                gt = pool.tile([P, ni], f32, tag="gt" + tagsfx)
                nc.vector.transpose(gt[:], g[:])
                mku = pool.tile([P, xs, 8], u8, tag="mku" + tagsfx, bufs=1)
                nc.scalar.dma_start(mku[:], mask8_ap)
                mk = pool.tile([P, xs, 8], f32, tag="mk" + tagsfx)
                nc.scalar.activation(mk[:], mku[:], Act.Copy)
                cand = gt[:].rearrange("p (c u q) -> p (c u) q", u=2, q=16)
                nc.vector.tensor_tensor(
                    out=mk[:], in0=mk[:], in1=cand[:, :, 0:8], op=Alu.mult)
                sel = pool.tile([P, xs], f32, tag="sel" + tagsfx, bufs=2)
                nc.vector.tensor_reduce(
                    out=sel[:], in_=mk[:],
                    axis=mybir.AxisListType.X, op=Alu.add)
                nc.vector.tensor_tensor(
                    out=s_ap[:, 0:xs], in0=s_ap[:, 0:xs], in1=sel[:],
                    op=Alu.add)

            # chase: trees interleaved two-wide
            cidx_t = {}
            for pair in (range(0, 2), range(2, 4)):
                for j in pair:
                    ci = pool.tile([P, k_iters, J], i16, tag=f"ci{j % 2}")
                    nc.scalar.dma_start(
                        ci[:], c_idx[j].rearrange("k p f -> p k f"))
                    cidx_t[j] = ci
                for k in range(k_iters):
                    ni = NI[k]
                    sfx = "" if ni == NE else f"n{ni}"
                    for j in pair:
                        tbl = rebuild_table(j, s_tiles[j][:], 0)
                        chase_round(tbl, cidx_t[j][:, k, 0:ni // 16],
                                    c_mask[j, k, :, 0:ni // 16, :],
                                    s_tiles[j][:], ni, sfx)

            # pixels: val = s/SCALING; core k serves chunk k, NR2 rounds
            for pair in (range(0, 2), range(2, 4)):
                pix_t = {}
                for j in pair:
                    nc.vector.tensor_scalar_mul(
                        s_tiles[j][:], s_tiles[j][:], 1.0 / cfg["SCALING"])
                    pi = pool.tile([P, NR2, NIPX // 16], i16, tag=f"pi{j % 2}")
                    nc.scalar.dma_start(
                        pi[:], p_idx[j].rearrange("k p f -> p k f"))
                    pix_t[j] = pi
                for j in pair:
                    tbl = rebuild_table(j, s_tiles[j][:], 1)
                    tblv = tbl[:].bitcast(f16)[:, 0:NE]
                    for R in range(NR2):
                        g2 = pool.tile([P, NE], f32, tag="g")
                        g2v = g2[:].bitcast(f16)
                        nc.gpsimd.ap_gather(
                            out_ap=g2v[:, 0:2 * NIPX], in_ap=tblv,
                            idxs_ap=pix_t[j][:, R, :],
                            channels=P, num_elems=NE // 2, d=2,
                            num_idxs=NIPX)
                        for q in range(8):
                            eng = nc.sync if q % 2 == 0 else nc.scalar
                            eng.dma_start(pixsorted[j, R, q:q + 1],
                                          g2v[16 * q:16 * q + 1, 0:2 * NIPX])
            _es.close()

    nc.compile()
    return nc


_CACHE = {}
TRACE = False
LAST_RESULT = None


def _get_nc(cfg, k_iters, geom):
    key = (k_iters, geom)
    if key not in _CACHE:
        _CACHE[key] = _build(cfg, k_iters, geom)
    return _CACHE[key]


def kernel(diff, attrs, weight, bias, parent, pix2cc):
    cfg = CFG
    diff = np.ascontiguousarray(np.asarray(diff, np.float32))
    attrs = np.ascontiguousarray(np.asarray(attrs, np.float32))
    weight = np.ascontiguousarray(np.asarray(weight, np.float32))
    bias = np.ascontiguousarray(np.asarray(bias, np.float32))
    parent = np.ascontiguousarray(np.asarray(parent, np.int32))
    pix2cc = np.ascontiguousarray(np.asarray(pix2cc, np.int32))

    in_maps, k_iters, geom = _host_prep(
        cfg, diff, attrs, weight, bias, parent, pix2cc)
    nc = _get_nc(cfg, k_iters, geom)
    res = run_bass_kernel_spmd(
        nc, in_maps, core_ids=list(range(cfg["NCORES"])), trace=TRACE)
    global LAST_RESULT
    LAST_RESULT = res
    return _host_assemble(cfg, res.results)
